# revision 1
# baseline (speedup 1.0000x reference)
"""CrossModalAttention TRN2 kernel v2: bf16 datapath + 4-row groups.

Same contract/sharding as kernel.py (v1). Differences:
 - Q/K/V evacuated to bf16 SBUF; QK products + tree-reduce mostly bf16
   (DVE 2x mode), final reduce stages fp32.
 - Attention processed in 4-token-row groups (FD 2048) to amortize DVE
   op overheads.
 - Attention weights (unnormalized exp) expanded over the 16 token
   pixels via SBUF->SBUF DMA (step-0 source dims) so the AV multiply
   runs at DVE 2x; normalization deferred to a final divide that also
   permutes back to pixel order.
"""

import os
from contextlib import ExitStack

import numpy as np

import concourse.bass as bass
import concourse.mybir as mybir
import concourse.tile as tile
from concourse.bass_utils import run_bass_kernel_spmd

B, C, H, W = 2, 256, 128, 128
TOK = 4
NH, NW = H // TOK, W // TOK
T2 = TOK * TOK
SCALE = float((C // T2) ** -0.5)
NCORES = 8
QH = 4
NH_LOC = NH // QH                    # 8 token rows / core
PIX_LOC = NH_LOC * TOK * W           # 4096
HALO_ROWS = NH_LOC + 2
PIX_HALO = HALO_ROWS * TOK * W       # 5120
ROWSZ = TOK * W                      # 512
G = 4                                # token rows per attention group
NG = NH_LOC // G                     # 2 groups
GSZ = G * ROWSZ                      # 2048 pixels per group
OFFS = [(di, dj) for di in (-1, 0, 1) for dj in (-1, 0, 1)]
NN = len(OFFS)

F32 = mybir.dt.float32
FP16 = mybir.dt.float16
BF16 = mybir.dt.bfloat16
AX = mybir.AxisListType
AF = mybir.ActivationFunctionType

# a-expand via DMA (step-0 src dims). Fallback: broadcast-AP multiply (1x).
EXPAND_DMA = os.environ.get("KERNEL2_NO_EXPAND") != "1"


def _build_kernel(nc: bass.Bass, ctx: ExitStack, tc: "tile.TileContext"):
    xb = nc.dram_tensor("xb", [C, PIX_LOC], FP16, kind="ExternalInput").ap()
    xw = nc.dram_tensor("xw", [C, PIX_HALO], FP16, kind="ExternalInput").ap()
    wq = nc.dram_tensor("wq", [C, C], FP16, kind="ExternalInput").ap()
    wk = nc.dram_tensor("wk", [C, C], FP16, kind="ExternalInput").ap()
    wv = nc.dram_tensor("wv", [C, C], FP16, kind="ExternalInput").ap()
    bq = nc.dram_tensor("bq", [2, 128, 1], F32, kind="ExternalInput").ap()
    bk = nc.dram_tensor("bk", [2, 128, 1], F32, kind="ExternalInput").ap()
    bv = nc.dram_tensor("bv", [2, 128, 1], F32, kind="ExternalInput").ap()
    mask = nc.dram_tensor("mask", [128, NH_LOC * NW * NN], BF16,
                          kind="ExternalInput").ap()
    out = nc.dram_tensor("out", [C, PIX_LOC], F32, kind="ExternalOutput").ap()

    const_pool = ctx.enter_context(tc.tile_pool(name="const", bufs=1))
    ps_pool = ctx.enter_context(tc.tile_pool(name="ps", bufs=4, space="PSUM"))
    qkv_pool = ctx.enter_context(tc.tile_pool(name="qkv", bufs=1))
    # ---- persistent constants
    w_sb = {}
    for name, wd in (("q", wq), ("k", wk), ("v", wv)):
        for ci in range(2):
            t = const_pool.tile([128, C], FP16, tag=f"w{name}{ci}",
                                name=f"w{name}{ci}")
            nc.sync.dma_start(t[:], wd[ci * 128:(ci + 1) * 128, :])
            w_sb[name, ci] = t
    b_sb = {}
    for name, bd in (("q", bq), ("k", bk), ("v", bv)):
        for co in range(2):
            t = const_pool.tile([128, 1], F32, tag=f"b{name}{co}",
                                name=f"b{name}{co}")
            nc.sync.dma_start(t[:], bd[co])
            b_sb[name, co] = t
    mask_sb = const_pool.tile([128, NH_LOC * NW * NN], BF16, tag="mask")
    nc.sync.dma_start(mask_sb[:], mask[:])

    # ACT/DVE warm-ups: same 1-sync-wait limit as PE (walrus setupSyncWait)
    # applies to Activation; cover the bias/mask DMAs on their consumer
    # engines before the real consumers run.
    scratch = const_pool.tile([128, 8], F32, tag="scratch", name="scratch")
    for wi, name in enumerate(("q", "k", "v")):
        for co in range(2):
            nc.scalar.activation(scratch[:, wi * 2 + co:wi * 2 + co + 1],
                                 b_sb[name, co][:],
                                 mybir.ActivationFunctionType.Identity,
                                 bias=b_sb[name, co][:])
    nc.vector.tensor_copy(scratch[:, 6:7], mask_sb[:, 0:1])

    # PE warm-up: cover every weight-DMA on PE's clock so real matmuls
    # carry at most one sync wait (walrus LDWEIGHTS limit).
    warm_ps = ps_pool.tile([128, 8], F32, tag="warm")
    for name in ("q", "k", "v"):
        for ci in range(2):
            nc.tensor.matmul(warm_ps[0:1, 0:1], w_sb[name, ci][:, 0:1],
                             w_sb[name, ci][:, 0:1], start=True, stop=True)

    q_sb = [qkv_pool.tile([128, PIX_LOC], BF16, tag=f"q{c}", name=f"q{c}")
            for c in range(2)]
    k_sb = [qkv_pool.tile([128, PIX_HALO + 2 * TOK], BF16, tag=f"k{c}",
                          name=f"k{c}") for c in range(2)]
    v_sb = [qkv_pool.tile([128, PIX_HALO + 2 * TOK], BF16, tag=f"v{c}",
                          name=f"v{c}") for c in range(2)]
    for t in (*k_sb, *v_sb):
        nc.vector.memset(t[:, 0:TOK], 0.0)
        nc.vector.memset(t[:, TOK + PIX_HALO:], 0.0)

    # ---- projections (fp32 matmul, bf16 evacuation via ACT)
    # x loaded into one-shot full buffers: no SBUF slot reuse means the DMA
    # triggers carry no WAW waits (walrus limits DMA triggers to 2 waits) and
    # dummy matmuls per 512-slice put every x-DMA queue on PE's clock so
    # real matmuls keep at most 1 wait.
    with tc.tile_pool(name="x", bufs=1) as x_pool:
        xb_sb = [x_pool.tile([128, PIX_LOC], FP16, tag=f"xb{ci}",
                             name=f"xb{ci}") for ci in range(2)]
        xw_sb = [x_pool.tile([128, PIX_HALO], FP16, tag=f"xw{ci}",
                             name=f"xw{ci}") for ci in range(2)]
        for ci in range(2):
            rows = slice(ci * 128, (ci + 1) * 128)
            for c0 in range(0, PIX_LOC, 1024):
                nc.sync.dma_start(xb_sb[ci][:, c0:c0 + 1024],
                                  xb[rows, c0:c0 + 1024])
            for c0 in range(0, PIX_HALO, 1024):
                nc.sync.dma_start(xw_sb[ci][:, c0:c0 + 1024],
                                  xw[rows, c0:c0 + 1024])
            for pt in range(PIX_LOC // 512):
                nc.tensor.matmul(warm_ps[0:1, 0:1],
                                 xb_sb[ci][:, pt * 512:pt * 512 + 1],
                                 xb_sb[ci][:, pt * 512:pt * 512 + 1],
                                 start=True, stop=True)
            for pt in range(PIX_HALO // 512):
                nc.tensor.matmul(warm_ps[0:1, 0:1],
                                 xw_sb[ci][:, pt * 512:pt * 512 + 1],
                                 xw_sb[ci][:, pt * 512:pt * 512 + 1],
                                 start=True, stop=True)

        def project(x_src, pix, projs):
            for pt in range(pix // 512):
                sl = slice(pt * 512, (pt + 1) * 512)
                for name, dst, scale, pad in projs:
                    dsl = slice(pad + pt * 512, pad + (pt + 1) * 512)
                    for co in range(2):
                        ps = ps_pool.tile([128, 512], F32)
                        co_sl = slice(co * 128, (co + 1) * 128)
                        if name == "v":
                            # stream V's rhs in (u,v,j) order so PSUM comes
                            # out in the [u,v,j] layout the AV stage wants
                            # and the evacuation is a contiguous copy
                            xs = [x[:, sl].rearrange(
                                "p (u j v) -> p u v j", u=TOK, j=NW, v=TOK)
                                for x in x_src]
                        else:
                            xs = [x[:, sl] for x in x_src]
                        nc.tensor.matmul(ps[:], w_sb[name, 0][:, co_sl],
                                         xs[0], start=True, stop=False)
                        nc.tensor.matmul(ps[:], w_sb[name, 1][:, co_sl],
                                         xs[1], start=False, stop=True)
                        nc.scalar.activation(dst[co][:, dsl], ps[:],
                                             AF.Identity,
                                             bias=b_sb[name, co][:],
                                             scale=scale)

        project(xb_sb, PIX_LOC, [("q", q_sb, SCALE, 0)])
        project(xw_sb, PIX_HALO, [("k", k_sb, 1.0, TOK),
                                  ("v", v_sb, 1.0, TOK)])

    # ---- attention in 4-row groups (pools allocated after x released)
    prod_pool = ctx.enter_context(tc.tile_pool(name="prod", bufs=3))
    tree_pool = ctx.enter_context(tc.tile_pool(name="tree", bufs=2))
    s_pool = ctx.enter_context(tc.tile_pool(name="s", bufs=2))
    e_pool = ctx.enter_context(tc.tile_pool(name="e", bufs=2))
    z_pool = ctx.enter_context(tc.tile_pool(name="z", bufs=2))
    ax_pool = ctx.enter_context(tc.tile_pool(name="ax", bufs=3))
    av_pool = ctx.enter_context(tc.tile_pool(name="av", bufs=6))
    acc_pool = ctx.enter_context(tc.tile_pool(name="acc", bufs=2))
    for ch in range(2):
        for g in range(NG):
            qsl = q_sb[ch][:, g * GSZ:(g + 1) * GSZ]
            GNJ = G * NW                      # 128 positions per group
            s_t = s_pool.tile([128, NN * GNJ], BF16, tag="s", name="s")
            # layout [n, i, j]: per-neighbor slice is contiguous
            s_v = s_t[:].rearrange("p (n i j) -> p n i j", n=NN, i=G, j=NW)
            for n, (di, dj) in enumerate(OFFS):
                koff = TOK + (g * G + 1 + di) * ROWSZ + dj * TOK
                prod = prod_pool.tile([128, GSZ], BF16, tag="prod",
                                      name="prod")
                qk_eng = nc.gpsimd if n in (4, 7) else nc.vector
                qk_eng.tensor_mul(prod[:], qsl,
                                  k_sb[ch][:, koff:koff + GSZ])
                # tree-reduce 16 pixels/token; every view kept <=3 free
                # dims (DVE TENSOR3D limit) by merging contiguous dims
                pv = prod[:].rearrange("p (iu j v) -> p iu j v",
                                       iu=G * TOK, j=NW, v=TOK)
                f1 = tree_pool.tile([128, GSZ // 2], BF16, tag="f1",
                                    name="f1")
                f1v = f1[:].rearrange("p (iu j v) -> p iu j v",
                                      iu=G * TOK, j=NW, v=2)
                nc.vector.tensor_add(f1v, pv[:, :, :, 0:2], pv[:, :, :, 2:4])
                f2 = tree_pool.tile([128, GSZ // 4], BF16, tag="f2",
                                    name="f2")
                f2v = f2[:].rearrange("p (i u jv) -> p i u jv",
                                      i=G, u=2, jv=2 * NW)
                f1u = f1[:].rearrange("p (i u jv) -> p i u jv",
                                      i=G, u=TOK, jv=2 * NW)
                nc.vector.tensor_add(f2v, f1u[:, :, 0:2], f1u[:, :, 2:4])
                f3 = tree_pool.tile([128, GSZ // 8], BF16, tag="f3",
                                    name="f3")
                f3v = f3[:].rearrange("p (i jv) -> p i jv", i=G, jv=2 * NW)
                nc.vector.tensor_add(f3v, f2v[:, :, 0], f2v[:, :, 1])
                f3s = f3[:].rearrange("p (i j v) -> p i j v",
                                      i=G, j=NW, v=2)
                nc.vector.tensor_add(s_v[:, n], f3s[:, :, :, 0],
                                     f3s[:, :, :, 1])
            # unnormalized masked exp weights (n-outer layout)
            e_t = e_pool.tile([128, NN * GNJ], BF16, tag="e", name="e")
            nc.scalar.activation(e_t[:], s_t[:], AF.Exp)
            em_t = e_pool.tile([128, NN * GNJ], BF16, tag="em", name="em")
            msl = mask_sb[:, g * NN * GNJ:(g + 1) * NN * GNJ]
            nc.gpsimd.tensor_mul(em_t[:], e_t[:], msl)
            em_v = em_t[:].rearrange("p (n i j) -> p n i j",
                                     n=NN, i=G, j=NW)
            z_t = z_pool.tile([128, GNJ], F32, tag="z", name="z")
            nc.vector.reduce_sum(
                z_t[:],
                em_t[:].rearrange("p (n ij) -> p ij n", n=NN, ij=GNJ),
                axis=AX.X)
            zr_t = z_pool.tile([128, GNJ], F32, tag="zr", name="zr")
            nc.vector.reciprocal(zr_t[:], z_t[:])

            # AV in (i,u,v,j) enumeration; a expanded over the 16 token
            # pixels by a rep-outer SBUF->SBUF DMA (innermost stays
            # contiguous, so DGE accepts it)
            stack = []
            for n, (di, dj) in enumerate(OFFS):
                voff = TOK + (g * G + 1 + di) * ROWSZ + dj
                vv = v_sb[ch][:, voff:voff + GSZ].rearrange(
                    "p (i uv j) -> p i uv j", i=G, uv=T2, j=NW)
                tmp = av_pool.tile([128, GSZ], BF16, tag="avt", name="avt")
                tv = tmp[:].rearrange("p (i uv j) -> p i uv j",
                                      i=G, uv=T2, j=NW)
                if EXPAND_DMA:
                    a_x = ax_pool.tile([128, GSZ], BF16, tag="ax", name="ax")
                    esl = em_t[:, n * GNJ:(n + 1) * GNJ]
                    nc.sync.dma_start(
                        a_x[:].rearrange("p (r f) -> p r f", r=T2, f=GNJ),
                        esl.unsqueeze(1).broadcast_to((128, T2, GNJ)))
                    axv = a_x[:].rearrange("p (uv i j) -> p i uv j",
                                           uv=T2, i=G, j=NW)
                    nc.vector.tensor_mul(tv, axv, vv)
                else:
                    an = em_v[:, n].unsqueeze(2).broadcast_to(
                        (128, G, T2, NW))
                    nc.vector.tensor_mul(tv, an, vv)
                stack.append((0, tmp))
                while len(stack) >= 2 and stack[-1][0] == stack[-2][0]:
                    l1, t1 = stack.pop()
                    l0, t0 = stack.pop()
                    r = av_pool.tile([128, GSZ], BF16, tag="avt", name="avr")
                    eng = nc.gpsimd if (l0 == 0 and n <= 3) else nc.vector
                    eng.tensor_add(r[:], t0[:], t1[:])
                    stack.append((l0 + 1, r))
            acc = acc_pool.tile([128, GSZ], BF16, tag="acc", name="acc")
            while len(stack) > 1:
                _, t1 = stack.pop()
                _, t0 = stack.pop()
                if stack:
                    r = av_pool.tile([128, GSZ], BF16, tag="avt", name="avr2")
                    nc.vector.tensor_add(r[:], t0[:], t1[:])
                    stack.append((99, r))
                else:
                    nc.vector.tensor_add(acc[:], t0[:], t1[:])
                    stack.append((99, acc))
            if stack[0][1] is not acc:
                nc.vector.tensor_copy(acc[:], stack[0][1][:])

            # divide by Z and permute back to pixel order. zr expanded
            # over u by DMA so the divide stays <=3 free dims:
            # zu[(i,u), j] = zr[i, j]
            zu = z_pool.tile([128, G * TOK * NW], F32, tag="zu", name="zu")
            nc.scalar.copy(
                zu[:].rearrange("p (i u j) -> p i u j", i=G, u=TOK, j=NW),
                zr_t[:].rearrange("p (i j) -> p i j", i=G, j=NW)
                .unsqueeze(2).broadcast_to((128, G, TOK, NW)))
            on = acc_pool.tile([128, GSZ], F32, tag="on", name="on")
            onv = on[:].rearrange("p (iu j v) -> p iu j v",
                                  iu=G * TOK, j=NW, v=TOK)
            accv = acc[:].rearrange("p (iu v j) -> p iu j v",
                                    iu=G * TOK, v=TOK, j=NW)
            zb = zu[:].rearrange("p (iu j) -> p iu j",
                                 iu=G * TOK, j=NW).unsqueeze(3) \
                .broadcast_to((128, G * TOK, NW, TOK))
            nc.vector.tensor_mul(onv, accv, zb)
            nc.sync.dma_start(
                out[ch * 128:(ch + 1) * 128, g * GSZ:(g + 1) * GSZ], on[:])


_CACHE = {}


# --- post-scheduling legalization: this walrus build rejects instructions
# with more sync wait/update commands than the ISA struct has slots; move
# the excess onto standalone EventSemaphore instructions (same mechanism as
# engine.wait_ge / the all-engine barrier).
WAIT_LIMIT = 1
UPDATE_LIMIT = 1


def _legalize_waits(nc):
    f = nc.m.functions[0]
    for blk in f.blocks:
        il = blk.instructions
        i = 0
        while i < len(il):
            ins = il[i]
            si = ins.sync_info
            if si is None or ins.opcode == "EventSemaphore":
                i += 1
                continue
            waits = list(si.on_wait)
            ups = list(si.on_update)
            changed = False
            if len(waits) > WAIT_LIMIT:
                excess, waits = waits[:-WAIT_LIMIT], waits[-WAIT_LIMIT:]
                for w in excess:
                    ev = mybir.InstEventSemaphore(
                        name=f"lgw-{nc.next_id()}", ins=[], outs=[])
                    ev.engine = ins.engine
                    ev.sync_info = mybir.SyncInfo(on_wait=[w], on_update=[])
                    il.insert(i, ev)
                    i += 1
            post = []
            if len(ups) > UPDATE_LIMIT:
                excess_u, ups = ups[UPDATE_LIMIT:], ups[:UPDATE_LIMIT]
                for u in excess_u:
                    ev = mybir.InstEventSemaphore(
                        name=f"lgu-{nc.next_id()}", ins=[], outs=[])
                    ev.engine = ins.engine
                    ev.sync_info = mybir.SyncInfo(on_wait=[], on_update=[u])
                    post.append(ev)
                changed = True
            if changed or len(list(si.on_wait)) > WAIT_LIMIT:
                ins.sync_info = mybir.SyncInfo(on_wait=waits, on_update=ups)
            for ev in post:
                i += 1
                il.insert(i, ev)
            i += 1


def _get_program():
    if "nc" not in _CACHE:
        nc = bass.Bass("TRN2", target_bir_lowering=False, debug=False)
        with tile.TileContext(nc) as tc:
            with ExitStack() as ctx:
                _build_kernel(nc, ctx, tc)
        if os.environ.get("KERNEL_NO_LEGALIZE") != "1":
            _legalize_waits(nc)
        _CACHE["nc"] = nc
    return _CACHE["nc"]


def _shard_inputs(blue_feat, white_feat, q_w, q_b, k_w, k_b, v_w, v_b):
    import ml_dtypes
    blue = np.ascontiguousarray(blue_feat, dtype=np.float16)
    white = np.ascontiguousarray(white_feat, dtype=np.float16)
    wts = {
        "wq": np.ascontiguousarray(np.asarray(q_w, np.float16).T),
        "wk": np.ascontiguousarray(np.asarray(k_w, np.float16).T),
        "wv": np.ascontiguousarray(np.asarray(v_w, np.float16).T),
        "bq": (np.asarray(q_b, np.float32) * SCALE).reshape(2, 128, 1).copy(),
        "bk": np.asarray(k_b, np.float32).reshape(2, 128, 1).copy(),
        "bv": np.asarray(v_b, np.float32).reshape(2, 128, 1).copy(),
    }
    in_maps = []
    for core in range(NCORES):
        b, q = divmod(core, QH)
        r0 = q * NH_LOC * TOK
        xb = blue[b, :, r0:r0 + NH_LOC * TOK, :].reshape(C, PIX_LOC)
        xw = np.zeros((C, HALO_ROWS * TOK, W), np.float16)
        lo, hi = r0 - TOK, r0 + (NH_LOC + 1) * TOK
        slo, shi = max(lo, 0), min(hi, H)
        xw[:, slo - lo:shi - lo, :] = white[b, :, slo:shi, :]
        xw = xw.reshape(C, PIX_HALO)
        gi = q * NH_LOC + np.arange(NH_LOC)[:, None, None]
        j = np.arange(NW)[None, :, None]
        di = np.array([o[0] for o in OFFS])[None, None, :]
        dj = np.array([o[1] for o in OFFS])[None, None, :]
        m = ((gi + di >= 0) & (gi + di < NH) &
             (j + dj >= 0) & (j + dj < NW)).astype(np.float32)
        # [i, j, n] -> [g, n, i_in_group, j] to match the kernel's n-outer
        # score layout
        m = m.reshape(NG, G, NW, NN).transpose(0, 3, 1, 2).reshape(-1)
        m = np.broadcast_to(m.reshape(1, -1),
                            (128, NH_LOC * NW * NN))
        m = m.astype(ml_dtypes.bfloat16).copy()
        in_maps.append({"xb": np.ascontiguousarray(xb),
                        "xw": np.ascontiguousarray(xw),
                        "mask": m, **wts})
    return in_maps


def _assemble(results):
    out = np.empty((B, C, H, W), np.float32)
    for core in range(NCORES):
        b, q = divmod(core, QH)
        r0 = q * NH_LOC * TOK
        out[b, :, r0:r0 + NH_LOC * TOK, :] = \
            results[core]["out"].reshape(C, NH_LOC * TOK, W)
    return out


def kernel(blue_feat, white_feat, q_w, q_b, k_w, k_b, v_w, v_b):
    nc = _get_program()
    in_maps = _shard_inputs(blue_feat, white_feat,
                            q_w, q_b, k_w, k_b, v_w, v_b)
    trace = os.environ.get("KERNEL_TRACE") == "1"
    res = run_bass_kernel_spmd(nc, in_maps, core_ids=list(range(NCORES)),
                               trace=trace)
    if trace:
        _CACHE["last_result"] = res
    return _assemble(res.results)



# revision 2
# speedup vs baseline: 1.0241x; 1.0241x over previous
"""CrossModalAttention TRN2 kernel v4: all-2x DVE datapath + PE AV-accumulate.

Changes vs v2 (kernel.py):
 - Q/K kept in (i,u,v,j) token layout so every QK tree-reduce level has a
   stride-1 innermost dim (all levels run at DVE 2x; v2 paid 1x on L4 and
   gpsimd offloads of 4x cost).
 - Attention weights normalized BEFORE the AV stage (an = exp*mask/Z), so
   the final divide + v<->j permute disappear entirely.
 - V kept in natural pixel layout; AV products come out pixel-ordered and
   the 9-neighbor sum runs on the PE via identity-matmul PSUM accumulation
   (replaces 8 DVE adds per slab with idle PE cycles).
 - Weight expansion over the 16 token pixels: v-broadcast via one gpsimd
   copy (stage A), u-broadcast folded into the AV multiply as a stride-0
   AP dim (innermost stays stride-1 so the multiply keeps DVE 2x).
 - Output stored bf16 (host upcasts); halves output DMA traffic.
 - No SBUF->SBUF broadcast DMA (v2 pushed 18.9MB / ~79k descriptors
   through all 16 queues).
"""

import os
from contextlib import ExitStack

import numpy as np

import concourse.bass as bass
import concourse.mybir as mybir
import concourse.tile as tile
from concourse.bass_utils import run_bass_kernel_spmd

B, C, H, W = 2, 256, 128, 128
TOK = 4
NH, NW = H // TOK, W // TOK          # 32, 32
T2 = TOK * TOK
SCALE = float((C // T2) ** -0.5)
NCORES = 8
QH = 4
NH_LOC = NH // QH                    # 8 token rows / core
ROWSZ = TOK * W                      # 512 px per token row
SLAB = NH_LOC * ROWSZ                # 4096 px per channel-half
HALO_ROWS = NH_LOC + 2
PIX_HALO = HALO_ROWS * ROWSZ         # 5120
NIJ = NH_LOC * NW                    # 256 tokens per core
OFFS = [(di, dj) for di in (-1, 0, 1) for dj in (-1, 0, 1)]
NN = len(OFFS)                       # 9
SNN = NN * NIJ                       # 2304 score slots

F32 = mybir.dt.float32
FP16 = mybir.dt.float16
BF16 = mybir.dt.bfloat16
AF = mybir.ActivationFunctionType

KPAD = 2                             # even pad: dj=0 reads stay 4B-aligned
VPAD = TOK                           # pixel layout: dj shift = +-4 elems
KSZ = KPAD + PIX_HALO + KPAD
K1SZ = PIX_HALO + 2                  # K shifted by one elem: dj=+-1 aligned
VSZ = VPAD + PIX_HALO + VPAD


def _build_kernel(nc: bass.Bass, ctx: ExitStack, tc: "tile.TileContext"):
    xb = nc.dram_tensor("xb", [C, SLAB], FP16, kind="ExternalInput").ap()
    xw = nc.dram_tensor("xw", [C, PIX_HALO], FP16, kind="ExternalInput").ap()
    wq = nc.dram_tensor("wq", [C, C], FP16, kind="ExternalInput").ap()
    wk = nc.dram_tensor("wk", [C, C], FP16, kind="ExternalInput").ap()
    wv = nc.dram_tensor("wv", [C, C], FP16, kind="ExternalInput").ap()
    bq = nc.dram_tensor("bq", [2, 128, 1], F32, kind="ExternalInput").ap()
    bk = nc.dram_tensor("bk", [2, 128, 1], F32, kind="ExternalInput").ap()
    bv = nc.dram_tensor("bv", [2, 128, 1], F32, kind="ExternalInput").ap()
    mask = nc.dram_tensor("mask", [128, SNN], BF16, kind="ExternalInput").ap()
    ident = nc.dram_tensor("ident", [128, 128], BF16,
                           kind="ExternalInput").ap()
    out = nc.dram_tensor("out", [C, SLAB], BF16, kind="ExternalOutput").ap()

    const_pool = ctx.enter_context(tc.tile_pool(name="const", bufs=1))

    # ---- persistent constants
    w_sb = {}
    for name, wd in (("q", wq), ("k", wk), ("v", wv)):
        for ci in range(2):
            t = const_pool.tile([128, C], FP16, tag=f"w{name}{ci}",
                                name=f"w{name}{ci}")
            nc.sync.dma_start(t[:], wd[ci * 128:(ci + 1) * 128, :])
            w_sb[name, ci] = t
    b_sb = {}
    for name, bd in (("q", bq), ("k", bk), ("v", bv)):
        for co in range(2):
            t = const_pool.tile([128, 1], F32, tag=f"b{name}{co}",
                                name=f"b{name}{co}")
            nc.sync.dma_start(t[:], bd[co])
            b_sb[name, co] = t
    mask_sb = const_pool.tile([128, SNN], BF16, tag="mask", name="mask")
    nc.sync.dma_start(mask_sb[:], mask[:])
    id_sb = const_pool.tile([128, 128], BF16, tag="ident", name="ident")
    nc.sync.dma_start(id_sb[:], ident[:])

    # ACT/DVE warm-ups: cover the bias/mask DMAs on their consumer engines
    # so the real consumers carry at most one sync wait (walrus limit).
    scratch = const_pool.tile([128, 8], F32, tag="scratch", name="scratch")
    for wi, name in enumerate(("q", "k", "v")):
        for co in range(2):
            nc.scalar.activation(scratch[:, wi * 2 + co:wi * 2 + co + 1],
                                 b_sb[name, co][:], AF.Identity,
                                 bias=b_sb[name, co][:])
    nc.vector.tensor_copy(scratch[:, 6:7], mask_sb[:, 0:1])

    # attention SBUF pools (allocated before x so x can be scoped/freed)
    qkv_pool = ctx.enter_context(tc.tile_pool(name="qkv", bufs=1))
    s_pool = ctx.enter_context(tc.tile_pool(name="s", bufs=2))
    an_pool = ctx.enter_context(tc.tile_pool(name="an", bufs=2))
    e_pool = ctx.enter_context(tc.tile_pool(name="e", bufs=1))
    z_pool = ctx.enter_context(tc.tile_pool(name="z", bufs=1))
    prod_pool = ctx.enter_context(tc.tile_pool(name="prod", bufs=1))
    tree_pool = ctx.enter_context(tc.tile_pool(name="tree", bufs=1))
    e1_pool = ctx.enter_context(tc.tile_pool(name="e1", bufs=2))
    av_pool = ctx.enter_context(tc.tile_pool(name="av", bufs=2))
    of_pool = ctx.enter_context(tc.tile_pool(name="of", bufs=2))

    q_sb = [qkv_pool.tile([128, SLAB], BF16, tag=f"q{c}", name=f"q{c}")
            for c in range(2)]
    k_sb = [qkv_pool.tile([128, KSZ], BF16, tag=f"k{c}", name=f"k{c}")
            for c in range(2)]
    k1_sb = [qkv_pool.tile([128, K1SZ], BF16, tag=f"k1{c}", name=f"k1{c}")
             for c in range(2)]
    v_sb = [qkv_pool.tile([128, VSZ], BF16, tag=f"v{c}", name=f"v{c}")
            for c in range(2)]
    for t in k_sb:
        nc.vector.memset(t[:, 0:KPAD], 0.0)
        nc.vector.memset(t[:, KPAD + PIX_HALO:], 0.0)
    for t in v_sb:
        nc.vector.memset(t[:, 0:VPAD], 0.0)
        nc.vector.memset(t[:, VPAD + PIX_HALO:], 0.0)

    with tc.tile_pool(name="ps", bufs=2, space="PSUM") as ps_pool:
        # PE warm-up: cover weight/ident DMAs on PE's clock.
        warm_ps = ps_pool.tile([128, 2048], F32, tag="ps", name="warm")
        for name in ("q", "k", "v"):
            for ci in range(2):
                nc.tensor.matmul(warm_ps[0:1, 0:1], w_sb[name, ci][:, 0:1],
                                 w_sb[name, ci][:, 0:1],
                                 start=True, stop=True)
        nc.tensor.matmul(warm_ps[0:1, 0:1], id_sb[:, 0:1], id_sb[:, 0:1],
                         start=True, stop=True)

        # ---- projections (fp32 matmul, bf16 evacuation via ACT)
        with tc.tile_pool(name="x", bufs=1) as x_pool:
            xb_sb = [x_pool.tile([128, SLAB], FP16, tag=f"xb{ci}",
                                 name=f"xb{ci}") for ci in range(2)]
            xw_sb = [x_pool.tile([128, PIX_HALO], FP16, tag=f"xw{ci}",
                                 name=f"xw{ci}") for ci in range(2)]
            for ci in range(2):
                rows = slice(ci * 128, (ci + 1) * 128)
                for c0 in range(0, SLAB, 1024):
                    nc.sync.dma_start(xb_sb[ci][:, c0:c0 + 1024],
                                      xb[rows, c0:c0 + 1024])
            for ci in range(2):
                rows = slice(ci * 128, (ci + 1) * 128)
                for c0 in range(0, PIX_HALO, 1024):
                    nc.sync.dma_start(xw_sb[ci][:, c0:c0 + 1024],
                                      xw[rows, c0:c0 + 1024])
                # dummy matmuls put every x-DMA queue on PE's clock
                for pt in range(SLAB // 512):
                    nc.tensor.matmul(warm_ps[:, 0:512],
                                     xb_sb[ci][:, pt * 512:pt * 512 + 128],
                                     xb_sb[ci][:, pt * 512:pt * 512 + 512],
                                     start=True, stop=True)
                for pt in range(PIX_HALO // 512):
                    nc.tensor.matmul(warm_ps[:, 0:512],
                                     xw_sb[ci][:, pt * 512:pt * 512 + 128],
                                     xw_sb[ci][:, pt * 512:pt * 512 + 512],
                                     start=True, stop=True)

            def project(name, co, src, dst, pix, pad, scale, token_order):
                for c0 in range(0, pix, 2048):
                    cw = min(2048, pix - c0)
                    ps = ps_pool.tile([128, 2048], F32, tag="ps")
                    for ci in range(2):
                        for s0 in range(0, cw, 512):
                            sl = slice(c0 + s0, c0 + s0 + 512)
                            rhs = src[ci][:, sl]
                            if token_order:
                                rhs = rhs.rearrange("p (u j v) -> p u v j",
                                                    u=TOK, j=NW, v=TOK)
                            nc.tensor.matmul(
                                ps[:, s0:s0 + 512],
                                w_sb[name, ci][:, co * 128:(co + 1) * 128],
                                rhs, start=(ci == 0), stop=(ci == 1))
                    nc.scalar.activation(dst[co][:, pad + c0:pad + c0 + cw],
                                         ps[:, 0:cw], AF.Identity,
                                         bias=b_sb[name, co][:], scale=scale)

            for co in range(2):
                project("q", co, xb_sb, q_sb, SLAB, 0, SCALE, True)
                project("k", co, xw_sb, k_sb, PIX_HALO, KPAD, 1.0, True)
                # K shifted one element left: dj=+-1 products read K1 at
                # even (4B-aligned) offsets, keeping DVE 2x
                nc.sync.dma_start(k1_sb[co][:], k_sb[co][:, 1:1 + K1SZ])
            for co in range(2):
                project("v", co, xw_sb, v_sb, PIX_HALO, VPAD, 1.0, False)

    # ---- attention, channel-interleaved: QK0, QK1, SM0+AV0, SM1+AV1
    # (exp/stage-A/PE-accum of one channel hide under the other's DVE work)
    ps_av = ctx.enter_context(tc.tile_pool(name="psav", bufs=2,
                                           space="PSUM"))
    s_ts = {}

    def qk_phase(ch):
        q = q_sb[ch]
        s_t = s_ts[ch] = s_pool.tile([128, SNN], BF16, tag="s",
                                     name=f"s{ch}")
        for n in (1, 4, 7, 0, 2, 3, 5, 6, 8):
            di, dj = OFFS[n]
            if dj == 0:
                ksrc, koff = k_sb[ch], KPAD + (1 + di) * ROWSZ
            else:
                # K1[y] = K[y-1] -> K[(1+di)*512 + dj + t] = K1[base + t]
                ksrc, koff = k1_sb[ch], (1 + di) * ROWSZ + dj + 1
            prod = prod_pool.tile([128, SLAB], BF16, tag="prod",
                                  name="prod")
            nc.vector.tensor_mul(prod[:], q[:],
                                 ksrc[:, koff:koff + SLAB])
            pv = prod[:].rearrange("p (iu v j) -> p iu v j",
                                   iu=NH_LOC * TOK, v=TOK, j=NW)
            f1 = tree_pool.tile([128, SLAB // 2], BF16, tag="f1",
                                name="f1")
            f1v = f1[:].rearrange("p (iu v j) -> p iu v j",
                                  iu=NH_LOC * TOK, v=2, j=NW)
            nc.vector.tensor_add(f1v, pv[:, :, 0:2, :], pv[:, :, 2:4, :])
            f2 = tree_pool.tile([128, SLAB // 4], BF16, tag="f2",
                                name="f2")
            f1u = f1[:].rearrange("p (i u vj) -> p i u vj",
                                  i=NH_LOC, u=TOK, vj=2 * NW)
            f2v = f2[:].rearrange("p (i u vj) -> p i u vj",
                                  i=NH_LOC, u=2, vj=2 * NW)
            nc.vector.tensor_add(f2v, f1u[:, :, 0:2], f1u[:, :, 2:4])
            f3 = tree_pool.tile([128, SLAB // 8], BF16, tag="f3",
                                name="f3")
            f3v = f3[:].rearrange("p (i vj) -> p i vj",
                                  i=NH_LOC, vj=2 * NW)
            nc.vector.tensor_add(f3v, f2v[:, :, 0], f2v[:, :, 1])
            f3j = f3[:].rearrange("p (i v j) -> p i v j",
                                  i=NH_LOC, v=2, j=NW)
            s_n = s_t[:, n * NIJ:(n + 1) * NIJ].rearrange(
                "p (i j) -> p i j", i=NH_LOC)
            nc.vector.tensor_add(s_n, f3j[:, :, 0, :], f3j[:, :, 1, :])

    def sm_av_phase(ch):
        s_t = s_ts[ch]
        # softmax with deferred normalization folded into the weights
        e_t = e_pool.tile([128, SNN], BF16, tag="e", name="e")
        nc.scalar.activation(e_t[:], s_t[:], AF.Exp)
        em_t = e_pool.tile([128, SNN], BF16, tag="em", name="em")
        nc.vector.tensor_mul(em_t[:], e_t[:], mask_sb[:])
        t1 = z_pool.tile([128, 1024], BF16, tag="t1", name="t1")
        nc.vector.tensor_add(t1[:], em_t[:, 0:1024], em_t[:, 1024:2048])
        t2 = z_pool.tile([128, 512], BF16, tag="t2", name="t2")
        nc.vector.tensor_add(t2[:], t1[:, 0:512], t1[:, 512:1024])
        t3 = z_pool.tile([128, 256], BF16, tag="t3", name="t3")
        nc.vector.tensor_add(t3[:], t2[:, 0:256], t2[:, 256:512])
        z_t = z_pool.tile([128, NIJ], F32, tag="z", name="z")
        nc.vector.tensor_add(z_t[:], t3[:], em_t[:, 2048:2304])
        zr_t = z_pool.tile([128, NIJ], F32, tag="zr", name="zr")
        nc.vector.reciprocal(zr_t[:], z_t[:])
        zrb = z_pool.tile([128, NIJ], BF16, tag="zrb", name="zrb")
        nc.vector.tensor_copy(zrb[:], zr_t[:])
        an_t = an_pool.tile([128, SNN], BF16, tag="an", name="an")
        em_v = em_t[:].rearrange("p (n ij) -> p n ij", n=NN)
        an_v = an_t[:].rearrange("p (n ij) -> p n ij", n=NN)
        zb = zrb[:].unsqueeze(1).broadcast_to((128, NN, NIJ))
        nc.vector.tensor_mul(an_v, em_v, zb)

        # AV: expand weights over v on ACT (stage A), multiply per pixel
        # row u (no stride-0 dims, keeps DVE 2x), accumulate the 9
        # neighbor terms in PSUM via identity matmuls.
        acc = [ps_av.tile([128, 2048], F32, tag="psav", name=f"acc{ch}{h}")
               for h in range(2)]
        for n, (di, dj) in enumerate(OFFS):
            voff = VPAD + (1 + di) * ROWSZ + dj * TOK
            e1 = e1_pool.tile([128, NIJ * TOK], BF16, tag="e1", name="e1")
            an_n = an_t[:, n * NIJ:(n + 1) * NIJ].rearrange(
                "p (i j) -> p i j", i=NH_LOC)
            e1v = e1[:].rearrange("p (i j v) -> p i j v",
                                  i=NH_LOC, j=NW)
            nc.scalar.copy(
                e1v, an_n.unsqueeze(3).broadcast_to(
                    (128, NH_LOC, NW, TOK)))
            avt = av_pool.tile([128, SLAB], BF16, tag="avt", name="avt")
            e1f = e1[:].rearrange("p (i jv) -> p i jv", i=NH_LOC)
            av_v = avt[:].rearrange("p (i u jv) -> p i u jv",
                                    i=NH_LOC, u=TOK, jv=TOK * NW)
            vv = v_sb[ch][:, voff:voff + SLAB].rearrange(
                "p (i u jv) -> p i u jv", i=NH_LOC, u=TOK)
            for u in range(TOK):
                nc.vector.tensor_mul(av_v[:, :, u], e1f, vv[:, :, u])
            for h in range(2):
                for sl in range(4):
                    c0 = (h * 4 + sl) * 512
                    nc.tensor.matmul(acc[h][:, sl * 512:(sl + 1) * 512],
                                     id_sb[:], avt[:, c0:c0 + 512],
                                     start=(n == 0), stop=(n == NN - 1))
        for h in range(2):
            out_f = of_pool.tile([128, 2048], BF16, tag="of", name="of")
            nc.scalar.copy(out_f[:], acc[h][:])
            nc.sync.dma_start(
                out[ch * 128:(ch + 1) * 128, h * 2048:(h + 1) * 2048],
                out_f[:])

    qk_phase(0)
    qk_phase(1)
    sm_av_phase(0)
    sm_av_phase(1)


_CACHE = {}


# --- post-scheduling legalization: this walrus build rejects instructions
# with more sync wait/update commands than the ISA struct has slots; move
# the excess onto standalone EventSemaphore instructions.
WAIT_LIMIT = 1
UPDATE_LIMIT = 1


def _legalize_waits(nc):
    f = nc.m.functions[0]
    for blk in f.blocks:
        il = blk.instructions
        i = 0
        while i < len(il):
            ins = il[i]
            si = ins.sync_info
            if si is None or ins.opcode == "EventSemaphore":
                i += 1
                continue
            waits = list(si.on_wait)
            ups = list(si.on_update)
            changed = False
            if len(waits) > WAIT_LIMIT:
                excess, waits = waits[:-WAIT_LIMIT], waits[-WAIT_LIMIT:]
                for w in excess:
                    ev = mybir.InstEventSemaphore(
                        name=f"lgw-{nc.next_id()}", ins=[], outs=[])
                    ev.engine = ins.engine
                    ev.sync_info = mybir.SyncInfo(on_wait=[w], on_update=[])
                    il.insert(i, ev)
                    i += 1
            post = []
            if len(ups) > UPDATE_LIMIT:
                excess_u, ups = ups[UPDATE_LIMIT:], ups[:UPDATE_LIMIT]
                for u in excess_u:
                    ev = mybir.InstEventSemaphore(
                        name=f"lgu-{nc.next_id()}", ins=[], outs=[])
                    ev.engine = ins.engine
                    ev.sync_info = mybir.SyncInfo(on_wait=[], on_update=[u])
                    post.append(ev)
                changed = True
            if changed or len(list(si.on_wait)) > WAIT_LIMIT:
                ins.sync_info = mybir.SyncInfo(on_wait=waits, on_update=ups)
            for ev in post:
                i += 1
                il.insert(i, ev)
            i += 1


def _get_program():
    if "nc" not in _CACHE:
        nc = bass.Bass("TRN2", target_bir_lowering=False, debug=False)
        with tile.TileContext(nc) as tc:
            with ExitStack() as ctx:
                _build_kernel(nc, ctx, tc)
        if os.environ.get("KERNEL_NO_LEGALIZE") != "1":
            _legalize_waits(nc)
        _CACHE["nc"] = nc
    return _CACHE["nc"]


def _shard_inputs(blue_feat, white_feat, q_w, q_b, k_w, k_b, v_w, v_b):
    import ml_dtypes
    blue = np.ascontiguousarray(blue_feat, dtype=np.float16)
    white = np.ascontiguousarray(white_feat, dtype=np.float16)
    wts = {
        "wq": np.ascontiguousarray(np.asarray(q_w, np.float16).T),
        "wk": np.ascontiguousarray(np.asarray(k_w, np.float16).T),
        "wv": np.ascontiguousarray(np.asarray(v_w, np.float16).T),
        "bq": (np.asarray(q_b, np.float32) * SCALE).reshape(2, 128, 1).copy(),
        "bk": np.asarray(k_b, np.float32).reshape(2, 128, 1).copy(),
        "bv": np.asarray(v_b, np.float32).reshape(2, 128, 1).copy(),
        "ident": np.eye(128, dtype=ml_dtypes.bfloat16),
    }
    in_maps = []
    for core in range(NCORES):
        b, qq = divmod(core, QH)
        r0 = qq * NH_LOC * TOK
        xb = blue[b, :, r0:r0 + NH_LOC * TOK, :].reshape(C, SLAB)
        xwp = np.zeros((C, HALO_ROWS * TOK, W), np.float16)
        lo, hi = r0 - TOK, r0 + (NH_LOC + 1) * TOK
        slo, shi = max(lo, 0), min(hi, H)
        xwp[:, slo - lo:shi - lo, :] = white[b, :, slo:shi, :]
        xwp = xwp.reshape(C, PIX_HALO)
        gi = qq * NH_LOC + np.arange(NH_LOC)[:, None, None]
        j = np.arange(NW)[None, :, None]
        di = np.array([o[0] for o in OFFS])[None, None, :]
        dj = np.array([o[1] for o in OFFS])[None, None, :]
        m = ((gi + di >= 0) & (gi + di < NH) &
             (j + dj >= 0) & (j + dj < NW)).astype(np.float32)
        # [i, j, n] -> [n, i, j] to match the kernel's n-outer score layout
        m = m.transpose(2, 0, 1).reshape(-1)
        m = np.broadcast_to(m.reshape(1, -1), (128, SNN))
        m = m.astype(ml_dtypes.bfloat16).copy()
        in_maps.append({"xb": np.ascontiguousarray(xb),
                        "xw": np.ascontiguousarray(xwp),
                        "mask": m, **wts})
    return in_maps


def _assemble(results):
    out = np.empty((B, C, H, W), np.float32)
    for core in range(NCORES):
        b, qq = divmod(core, QH)
        r0 = qq * NH_LOC * TOK
        out[b, :, r0:r0 + NH_LOC * TOK, :] = \
            np.asarray(results[core]["out"]).astype(np.float32) \
            .reshape(C, NH_LOC * TOK, W)
    return out


def kernel(blue_feat, white_feat, q_w, q_b, k_w, k_b, v_w, v_b):
    nc = _get_program()
    in_maps = _shard_inputs(blue_feat, white_feat,
                            q_w, q_b, k_w, k_b, v_w, v_b)
    trace = os.environ.get("KERNEL_TRACE") == "1"
    res = run_bass_kernel_spmd(nc, in_maps, core_ids=list(range(NCORES)),
                               trace=trace)
    if trace:
        _CACHE["last_result"] = res
    return _assemble(res.results)
